# revision 10
# baseline (speedup 1.0000x reference)
"""Cross-attention (B=4, NQ=1024, P=2048, D=1024, H=16) on 8 trn2 NeuronCores.

Sharding: data-parallel over batch (4) x query-rows (2): core c handles
batch c//2, query rows (c%2)*512:(c%2)*512+512.  Each core runs the full
pipeline locally (K/V projections are duplicated within a batch pair), so
no collectives are needed and LayerNorm is fully local.

v2 changes vs the first working kernel (633us device time):
  * bf16 for the DMA-heavy inputs (qT, CT, Wq/Wk/Wv images): halves the
    front-of-kernel DMA bytes.  Attention operands (Q^T/K^T/V/P/O^T/Wo)
    stay f32r so the softmax path keeps ~1e-4 rounding.
  * VA ones-column comes from vector.memset, not a 4-byte-packet broadcast
    DMA (the old one issued 8192 tiny SWDGE packets per pass and starved
    the CT load).
  * bq/bk ship as one host-packed [128, 16] tile (single DMA) instead of
    two 1024-packet rearrange DMAs.
  * CT loads in 4 key-chunks on the SWDGE ring so K-proj group pc only
    waits for its own chunk; front weight loads are spread across BOTH
    HWDGE queues (sync q1 was previously idle until the output store).
  * Q projection is chunked per pass: chunk 0 up front, chunk X+1
    interleaved into pass-X attention together with the K^T/V projections
    for pass X+1, so the PE never drains (keeps the HAM clock at 2.4GHz).
  * Wo chunks 0/1 prefetch during pass-3 attention on both HWDGE rings.
"""

import os
import sys

for _p in ("/opt/trn_rl_repo", "/root/.axon_site/_ro/trn_rl_repo"):
    if os.path.isdir(_p) and _p not in sys.path:
        sys.path.insert(0, _p)

import numpy as np

import concourse.bass as bass
import concourse.mybir as mybir
import concourse.tile as tile
from concourse import bacc

F32 = mybir.dt.float32
F32R = mybir.dt.float32r
BF16 = mybir.dt.bfloat16
AF = mybir.ActivationFunctionType
OP = mybir.AluOpType

B, NQ, P, D, H, DK = 4, 1024, 2048, 1024, 16, 64
EPS = 1e-5
NQS = NQ // 2          # query rows per core
NT = D // 128          # 8 tiles over D
NKT = P // 128         # 16 tiles over keys
NPASS = 4              # head-quarter passes
HPP = H // NPASS       # 4 heads per pass
SCALE = 1.0 / np.sqrt(DK)


def _bcast(ap, parts=128):
    """DRAM 1-D tensor -> [parts, n] broadcast AP (partition step 0)."""
    return bass.AP(tensor=ap.tensor, offset=ap.offset, ap=[[0, parts]] + list(ap.ap))


def _build(repeat=1):
    nc = bacc.Bacc(None, target_bir_lowering=False)

    qT = nc.dram_tensor("qT", [128, NT, NQS], BF16, kind="ExternalInput")
    CT = nc.dram_tensor("CT", [128, NT, P], BF16, kind="ExternalInput")
    WqT = nc.dram_tensor("WqT", [4, 128, NT, 256], BF16, kind="ExternalInput")
    WkT = nc.dram_tensor("WkT", [4, 128, NT, 256], BF16, kind="ExternalInput")
    # Wv chunks carry a 65th all-zero column per head whose bias is 1.0, so
    # V-proj emits the softmax-denominator ones column directly into VA.
    WvT = nc.dram_tensor("WvT", [4, 128, NT, 260], BF16, kind="ExternalInput")
    WoT = nc.dram_tensor("WoT", [4, 128, NT, 256], F32R, kind="ExternalInput")
    bqkT = nc.dram_tensor("bqkT", [128, 16], F32, kind="ExternalInput")
    bva = nc.dram_tensor("bva", [H * (DK + 1)], F32, kind="ExternalInput")
    bo = nc.dram_tensor("bo", [D], F32, kind="ExternalInput")
    lnw = nc.dram_tensor("lnw", [D], F32, kind="ExternalInput")
    lnb = nc.dram_tensor("lnb", [D], F32, kind="ExternalInput")
    ones64 = nc.dram_tensor("ones64", [DK], F32R, kind="ExternalInput")
    out = nc.dram_tensor("out", [NQS, D], F32, kind="ExternalOutput")

    with tile.TileContext(nc) as tc:
        with (
            tc.tile_pool(name="const", bufs=1) as const,
            tc.tile_pool(name="big", bufs=1) as big,
            tc.tile_pool(name="w", bufs=3) as wp,
            tc.tile_pool(name="pt", bufs=3) as ptp,
            tc.tile_pool(name="yo", bufs=1) as yop,
            tc.tile_pool(name="misc", bufs=1) as misc,
            tc.tile_pool(name="ps", bufs=4, space="PSUM") as psp,
            tc.tile_pool(name="oa", bufs=2, space="PSUM") as oap,
            tc.tile_pool(name="bc", bufs=1, space="PSUM") as bcp,
        ):
            for _ in range(repeat):
                _emit(nc, const, big, wp, ptp, yop, misc,
                      psp, oap, bcp,
                      qT, CT, WqT, WkT, WvT, WoT,
                      bqkT, bva, bo, lnw, lnb, ones64, out)
    nc.finalize()
    return nc


def _emit(nc, const, big, wp, ptp, yop, misc,
          psp, oap, bcp,
          qT, CT, WqT, WkT, WvT, WoT,
          bqkT, bva, bo, lnw, lnb, ones64, out):
    # ---- tiny constants (sync ring, ahead of the weight loads) -----
    bqk = const.tile([128, 16], F32, tag="bqk")
    nc.sync.dma_start(out=bqk, in_=bqkT[:, :])
    ones_sb = const.tile([1, DK], F32R, tag="ones")
    nc.sync.dma_start(out=ones_sb, in_=ones64[None, :])
    eps_sb = const.tile([128, 1], F32, tag="eps")
    nc.vector.memset(eps_sb, EPS)

    # ---- persistent activations -----------------------------------
    QT_sb = big.tile([128, NT, NQS], F32R, tag="qt")    # Q^T, all heads
    OT_sb = big.tile([128, NT, NQS], F32R, tag="ot")    # O^T, all heads
    CTres = big.tile([128, NT, P], BF16, tag="ct")      # C^T resident
    qTs = big.tile([128, NT, NQS], BF16, tag="qts")     # q^T resident

    # CT in 4 key-chunks on SWDGE: K-proj group pc waits only chunk pc.
    for pc in (0, 1):
        nc.gpsimd.dma_start(out=CTres[:, :, pc * 512:(pc + 1) * 512],
                            in_=CT[:, :, pc * 512:(pc + 1) * 512])
    bvb = const.tile([128, H * (DK + 1)], F32, tag="bcst", bufs=3, name="bvb")
    nc.gpsimd.dma_start(out=bvb, in_=_bcast(bva[:]))
    for pc in (2, 3):
        nc.gpsimd.dma_start(out=CTres[:, :, pc * 512:(pc + 1) * 512],
                            in_=CT[:, :, pc * 512:(pc + 1) * 512])

    nc.scalar.dma_start(out=qTs, in_=qT[:, :, :])

    # ---- Q projection, one 256-wide chunk per pass ----------------
    def load_wq(c):
        w = wp.tile([128, NT, 256], BF16, tag="w", name=f"wq{c}")
        nc.sync.dma_start(out=w, in_=WqT[c, :, :, :])
        return w

    def qproj_groups(c, wq):
        for t2 in range(2):
            def qgroup(t2=t2):
                t = c * 2 + t2
                ps = psp.tile([128, NQS], F32, tag="ps")
                for dt in range(NT):
                    nc.tensor.matmul(
                        ps,
                        wq[:, dt, t2 * 128:(t2 + 1) * 128],
                        qTs[:, dt, :],
                        start=(dt == 0),
                        stop=(dt == NT - 1),
                    )
                nc.vector.tensor_scalar_add(QT_sb[:, t, :], ps, bqk[:, t:t + 1])
            yield qgroup

    # ---- per-pass K^T / V_aug projection machinery ----------------
    KT = [None] * NPASS
    VA = [None] * NPASS

    def open_pass(X):
        """Allocate pass buffers + weight loads; returns wk/wv tiles."""
        KT[X] = big.tile([128, 2, P], F32R, tag=f"kt{X % 2}", name=f"KTp{X}")
        VA[X] = big.tile([128, NKT, HPP, DK + 1], F32R, tag=f"va{X % 2}", name=f"VAp{X}")
        wk = wp.tile([128, NT, 256], BF16, tag="w", name=f"wk{X}")
        nc.sync.dma_start(out=wk, in_=WkT[X, :, :, :])
        wv = wp.tile([128, NT, 260], BF16, tag="w", name=f"wv{X}")
        nc.scalar.dma_start(out=wv, in_=WvT[X, :, :, :])
        return wk, wv

    def proj_groups(X, wk, wv):
        """Generator of emit-callables: one PE psum-group (8 MMs) each.

        K^T: 2 do-tiles x 4 p-chunks (N=512) = 8 groups;
        V: 16 k-tiles (N=256) = 16 groups.  24 groups per pass.
        """
        hb = X * HPP * (DK + 1)
        for pc in range(P // 512):
            for t2 in range(2):
                def kgroup(t2=t2, pc=pc):
                    ps = psp.tile([128, 512], F32, tag="ps")
                    for dt in range(NT):
                        nc.tensor.matmul(
                            ps,
                            wk[:, dt, t2 * 128:(t2 + 1) * 128],
                            CTres[:, dt, pc * 512:(pc + 1) * 512],
                            start=(dt == 0),
                            stop=(dt == NT - 1),
                        )
                    tglob = X * 2 + t2
                    nc.vector.tensor_scalar_add(
                        KT[X][:, t2, pc * 512:(pc + 1) * 512], ps,
                        bqk[:, 8 + tglob:9 + tglob])
                yield kgroup
        for kt in range(NKT):
            def vgroup(kt=kt):
                ps = psp.tile([128, HPP * (DK + 1)], F32, tag="ps")
                for dt in range(NT):
                    nc.tensor.matmul(
                        ps,
                        CTres[:, dt, kt * 128:(kt + 1) * 128],
                        wv[:, dt, :],
                        start=(dt == 0),
                        stop=(dt == NT - 1),
                    )
                nc.vector.tensor_add(
                    VA[X][:, kt, :, :],
                    ps.rearrange("p (h d) -> p h d", h=HPP),
                    bvb[:, hb:hb + HPP * (DK + 1)].rearrange(
                        "p (h d) -> p h d", h=HPP),
                )
            yield vgroup

    _tail = [None]

    def _flush_tail():
        if _tail[0] is not None:
            _tail[0]()
            _tail[0] = None

    def attention_head(X, hh, gen):
        """One head's S^T/exp/PV chain, interleaving proj groups of X+1.

        S/exp run 2 iterations ahead of PV so the PE never waits on the
        ACT exp latency (PE issue order: S0 S1 S2 PV0 S3 PV1 ...).
        """
        h = X * HPP + hh
        tloc, prow = hh // 2, (hh % 2) * DK
        tq, qrow = h // 2, (h % 2) * DK
        oa = oap.tile([DK + 1, NQS], F32, tag="oa")

        def s_exp(kt):
            sps = psp.tile([128, NQS], F32, tag="ps")
            nc.tensor.matmul(
                sps,
                KT[X][prow:prow + DK, tloc, kt * 128:(kt + 1) * 128],
                QT_sb[qrow:qrow + DK, tq, :],
                start=True, stop=True,
            )
            pt = ptp.tile([128, NQS], F32R, tag="pt")
            nc.scalar.activation(pt, sps, AF.Exp, scale=float(SCALE))
            return pt

        pts = {0: s_exp(0), 1: s_exp(1)}
        _flush_tail()      # previous head's normalization, off the hot path
        for kt in range(NKT):
            if kt + 2 < NKT:
                pts[kt + 2] = s_exp(kt + 2)
            nc.tensor.matmul(
                oa,
                VA[X][:, kt, hh, :],
                pts.pop(kt),
                start=(kt == 0),
                stop=(kt == NKT - 1),
            )
            if gen is not None and kt % 2 == 1:
                g = next(gen, None)
                if g is not None:
                    g()

        def tail(oa=oa, tq=tq, qrow=qrow):
            rc = misc.tile([1, NQS], F32R, tag="rc")
            with nc.allow_low_precision(reason="f32r keeps ~19 mantissa bits"):
                nc.vector.reciprocal(rc, oa[DK:DK + 1, :])
            bc = bcp.tile([DK, NQS], F32, tag="bc")
            nc.tensor.matmul(bc, ones_sb, rc, start=True, stop=True)
            bcs = misc.tile([DK, NQS], F32R, tag="bcs")
            nc.vector.tensor_copy(bcs, bc)
            nc.vector.tensor_mul(
                OT_sb[qrow:qrow + DK, tq, :], oa[0:DK, :], bcs)

        _tail[0] = tail

    # ---- o_proj prefetch machinery --------------------------------
    wo_tiles = {}

    def get_wo(c):
        if c not in wo_tiles and c < 4:
            w = wp.tile([128, NT, 256], F32R, tag="w", name=f"wo{c}")
            eng = nc.sync if c % 2 == 0 else nc.scalar
            eng.dma_start(out=w, in_=WoT[c, :, :, :])
            wo_tiles[c] = w
        return wo_tiles.get(c)

    def wo_prefetch_gen():
        yield lambda: get_wo(0)
        yield lambda: get_wo(1)
        def load_lnbb():
            lnbb = const.tile([128, D], F32, tag="bcst", bufs=3, name="lnbb")
            nc.gpsimd.dma_start(out=lnbb, in_=_bcast(lnb[:]))
            wo_tiles["lnbb"] = lnbb
        yield load_lnbb

    # ---- front: Q chunk 0, then pass-0 projections ----------------
    wq0 = load_wq(0)
    for g in qproj_groups(0, wq0):
        g()
    wk0, wv0 = open_pass(0)
    for g in proj_groups(0, wk0, wv0):
        g()

    # bob/lnwb on SWDGE once CT is through (needed only at o_proj/LN)
    bob = const.tile([128, D], F32, tag="bcst", bufs=3, name="bob")
    nc.gpsimd.dma_start(out=bob, in_=_bcast(bo[:]))
    lnwb = const.tile([128, D], F32, tag="bcst", bufs=3, name="lnwb")
    nc.gpsimd.dma_start(out=lnwb, in_=_bcast(lnw[:]))

    import itertools
    for X in range(NPASS):
        if X + 1 < NPASS:
            wqn = load_wq(X + 1)
            wkn, wvn = open_pass(X + 1)
            gen = itertools.chain(
                qproj_groups(X + 1, wqn), proj_groups(X + 1, wkn, wvn))
        else:
            gen = wo_prefetch_gen()
        for hh in range(HPP):
            attention_head(X, hh, gen)
        for g in gen:   # leftovers
            g()
    _flush_tail()

    # ---- o_proj: Yo[q, do] = O @ Wo^T + bo ------------------------
    # (yo_all shares the kt0 tag slot: KT pass-2 is dead by o_proj time)
    yo_all = big.tile([128, NQS // 128, D], F32, tag="kt0", name="yo_all")
    for doc in range(4):
        wo = get_wo(doc)
        for qt in range(NQS // 128):
            ps = psp.tile([128, 256], F32, tag="ps")
            for dt in range(NT):
                nc.tensor.matmul(
                    ps,
                    OT_sb[:, dt, qt * 128:(qt + 1) * 128],
                    wo[:, dt, :],
                    start=(dt == 0),
                    stop=(dt == NT - 1),
                )
            nc.vector.tensor_add(
                yo_all[:, qt, doc * 256:(doc + 1) * 256], ps,
                bob[:, doc * 256:(doc + 1) * 256])
        get_wo(doc + 2)   # prefetch; slot of wo[doc] frees after this doc

    # ---- LayerNorm over do, per 128-row q tile --------------------
    lnbb = wo_tiles["lnbb"]
    for qt in range(NQS // 128):
        row = yo_all[:, qt, :]
        stats = misc.tile([128, 2, 6], F32, tag="stats")
        row2 = row.rearrange("p (s n) -> p s n", s=2)
        for s in range(2):
            nc.vector.bn_stats(stats[:, s, :], row2[:, s, :])
        mv = misc.tile([128, 2], F32, tag="mv")
        nc.vector.bn_aggr(mv, stats)
        std = misc.tile([128, 1], F32, tag="std")
        nc.scalar.activation(std, mv[:, 1:2], AF.Sqrt, bias=eps_sb)
        rstd = misc.tile([128, 1], F32, tag="rstd")
        nc.vector.reciprocal(rstd, std)
        nc.vector.tensor_scalar(row, row, mv[:, 0:1], rstd,
                                OP.subtract, OP.mult)
        nc.vector.tensor_mul(row, row, lnwb)
        ob = yop.tile([128, D], F32, tag="ob")
        nc.vector.tensor_add(ob, row, lnbb)
        nc.sync.dma_start(out=out[qt * 128:(qt + 1) * 128, :], in_=ob)


# ---------------------------------------------------------------------------
# host side: cached PJRT runner (same machinery run_bass_kernel_spmd uses
# under axon, but the jitted executable is built once and reused)
# ---------------------------------------------------------------------------
_CACHE = {}


class _Runner:
    def __init__(self, nc, n_cores=8, donate=True):
        import jax
        from jax.experimental.shard_map import shard_map
        from jax.sharding import Mesh, PartitionSpec

        from concourse import bass2jax

        bass2jax.install_neuronx_cc_hook()
        self.jax = jax
        self.n_cores = n_cores
        partition_name = (
            nc.partition_id_tensor.name if nc.partition_id_tensor else None)
        in_names, out_names, out_avals = [], [], []
        for alloc in nc.m.functions[0].allocations:
            if not isinstance(alloc, mybir.MemoryLocationSet):
                continue
            name = alloc.memorylocations[0].name
            if alloc.kind == "ExternalInput":
                if name != partition_name:
                    in_names.append(name)
            elif alloc.kind == "ExternalOutput":
                out_names.append(name)
                out_avals.append(jax.core.ShapedArray(
                    tuple(alloc.tensor_shape), mybir.dt.np(alloc.dtype)))
        self.param_names = in_names
        self.out_names = out_names
        self.out_avals = out_avals
        n_params = len(in_names)
        all_in = list(in_names) + list(out_names)
        if partition_name is not None:
            all_in.append(partition_name)

        def _body(*args):
            operands = list(args)
            if partition_name is not None:
                operands.append(bass2jax.partition_id_tensor())
            return tuple(bass2jax._bass_exec_p.bind(
                *operands,
                out_avals=tuple(out_avals),
                in_names=tuple(all_in),
                out_names=tuple(out_names),
                lowering_input_output_aliases=(),
                sim_require_finite=True,
                sim_require_nnan=True,
                nc=nc,
            ))

        devices = jax.devices()[:n_cores]
        self.mesh = Mesh(np.asarray(devices), ("core",))
        donate_idx = (
            tuple(range(n_params, n_params + len(out_names))) if donate else ())
        in_specs = (PartitionSpec("core"),) * (n_params + len(out_names))
        out_specs = (PartitionSpec("core"),) * len(out_names)
        self.fn = jax.jit(
            shard_map(_body, mesh=self.mesh, in_specs=in_specs,
                      out_specs=out_specs, check_rep=False),
            donate_argnums=donate_idx, keep_unused=True)

    def concat_inputs(self, in_maps):
        return [
            np.concatenate([np.asarray(m[n]) for m in in_maps], axis=0)
            for n in self.param_names
        ]

    def zeros(self):
        return [
            np.zeros((self.n_cores * a.shape[0], *a.shape[1:]), a.dtype)
            for a in self.out_avals
        ]

    def run_concat(self, concat_in, zeros=None):
        if zeros is None:
            zeros = self.zeros()
        outs = self.fn(*concat_in, *zeros)
        self.jax.block_until_ready(outs)
        return outs

    def __call__(self, in_maps):
        outs = self.run_concat(self.concat_inputs(in_maps))
        res = []
        for c in range(self.n_cores):
            res.append({
                name: np.asarray(outs[i]).reshape(
                    self.n_cores, *self.out_avals[i].shape)[c]
                for i, name in enumerate(self.out_names)
            })
        return res


def _get_runner(repeat=1, donate=True):
    key = (repeat, donate)
    if key not in _CACHE:
        _CACHE[key] = _Runner(_build(repeat), donate=donate)
    return _CACHE[key]


def _bf16(x):
    import ml_dtypes
    return np.ascontiguousarray(np.asarray(x).astype(ml_dtypes.bfloat16))


def _sbuf_image(mat2d):
    """[D, n] -> [128, NT, n] SBUF image (partition-major, contiguous)."""
    d, n = mat2d.shape
    return np.ascontiguousarray(
        mat2d.reshape(d // 128, 128, n).transpose(1, 0, 2))


def _w_image(w):
    """torch-Linear weight [do, di] -> [4, 128, NT, 256] chunked W^T image."""
    wt = np.asarray(w, np.float32).T      # [di, do]
    chunks = [_sbuf_image(wt[:, c * 256:(c + 1) * 256]) for c in range(4)]
    return np.ascontiguousarray(np.stack(chunks, axis=0))


def make_in_maps(q, C, Wq, bq, Wk, bk, Wv, bv, Wo, bo, ln_w, ln_b):
    f32 = lambda x: np.ascontiguousarray(np.asarray(x, dtype=np.float32))
    q, C = f32(q), f32(C)
    WqT, WkT = (_bf16(_w_image(w)) for w in (Wq, Wk))
    WoT = _w_image(Wo)
    bq, bk, bv, bo, ln_w, ln_b = map(f32, (bq, bk, bv, bo, ln_w, ln_b))
    # Wv^T with a zero 65th column per head (bias 1.0 -> VA ones column)
    wvt = np.asarray(Wv, np.float32).T.reshape(D, H, DK)   # [di, h, dk]
    wva = np.concatenate(
        [wvt, np.zeros((D, H, 1), np.float32)], axis=2).reshape(D, H * (DK + 1))
    WvT = _bf16(np.stack(
        [_sbuf_image(wva[:, c * 260:(c + 1) * 260]) for c in range(4)], axis=0))
    bva = np.concatenate(
        [np.concatenate([bv.reshape(H, DK),
                         np.ones((H, 1), np.float32)], axis=1).reshape(-1)])
    bva = np.ascontiguousarray(bva)
    bqkT = np.stack(
        [bq[j * 128:(j + 1) * 128] for j in range(8)]
        + [bk[j * 128:(j + 1) * 128] for j in range(8)], axis=1)
    bqkT = np.ascontiguousarray(bqkT)
    ones = np.ones(DK, np.float32)
    CTs = [_bf16(_sbuf_image(np.ascontiguousarray(C[b].T))) for b in range(B)]
    in_maps = []
    for c in range(8):
        b, qh = c // 2, c % 2
        qTs = _bf16(_sbuf_image(
            np.ascontiguousarray(q[b, qh * NQS:(qh + 1) * NQS, :].T)))
        in_maps.append({
            "qT": qTs, "CT": CTs[b],
            "WqT": WqT, "WkT": WkT, "WvT": WvT, "WoT": WoT,
            "bqkT": bqkT, "bva": bva, "bo": bo,
            "lnw": ln_w, "lnb": ln_b, "ones64": ones,
        })
    return in_maps


def kernel(q, C, Wq, bq, Wk, bk, Wv, bv, Wo, bo, ln_w, ln_b):
    in_maps = make_in_maps(q, C, Wq, bq, Wk, bk, Wv, bv, Wo, bo, ln_w, ln_b)
    res = _get_runner(1)(in_maps)
    out = np.empty((B, NQ, D), dtype=np.float32)
    for c in range(8):
        b, qh = c // 2, c % 2
        out[b, qh * NQS:(qh + 1) * NQS, :] = res[c]["out"]
    return out
